# revision 15
# baseline (speedup 1.0000x reference)
"""Cross-attention kernel for Trainium2, 8 NeuronCores, data-parallel over batch.

Reference computation (per batch b):
  lq = Wl @ lb + bl          [D, N]   (D == N == 256)
  fk = Wf @ fm + bf          [D, N]
  v  = Wv @ fm + bv          [C, N]
  att = softmax(lq @ fk.T)   [D, D]   (softmax over last dim)
  out = v @ att.T + fm       [C, N]

Strategy: batch 64 is split 8 ways (8 batches per core). All matmuls are
emitted with the activation tile as the stationary operand (lhsT) and the
weights streaming, computing transposed projections directly:
  fkT[n, d] = sum_c fm[c, n] WfT[c, d]     (fp32r / FP22)
  lqT[n, d] = sum_l lb[l, n] WlT[l, d]     (fp32r)
  logits[d, e] = sum_n lqT[n, d] fkT[n, e] (fp32r)
  vT[n, c] = sum_c' fm[c', n] WvT[c', c]   (bf16)
  out[c, d] = sum_n vT[n, c] attT[n, d]    (bf16)
Biases bl/bf enter via rank-1 augmentation matmuls (ones x bias row); bv is
added in the epilogue (exact: softmax rows sum to 1). Residual add is fused
into the PSUM->SBUF epilogue.
"""

import numpy as np
import ml_dtypes

import concourse.bass as bass
import concourse.mybir as mybir
import concourse.tile as tile
from concourse import bacc
from concourse.bass_utils import run_bass_kernel_spmd
from concourse.masks import make_identity

N_CORES = 8
B = 64
C = 2048
L = 512
HW = 256          # N = H*W, == D
D = HW
P = 128
B_SHARD = B // N_CORES

F32 = mybir.dt.float32
F32R = mybir.dt.float32r
BF16 = mybir.dt.bfloat16

CK = C // P       # 16 k-tiles over channel contraction
LK = L // P       # 4 k-tiles over label contraction
NT = HW // P      # 2 tiles over spatial/projection dim
CM = C // P       # 16 output-channel chunks
VC = 512          # value-matmul free-dim chunk (one PSUM bank of fp32)


def build_kernel(b_shard=B_SHARD):
    nc = bacc.Bacc("TRN2", target_bir_lowering=False, debug=False,
                   num_devices=N_CORES)

    # all tensors host-pre-transposed to [partition, ktile, free] so each DMA
    # is 128 large contiguous lines (descriptor-generation cost on the issuing
    # engine is ~2.4ns/line; fragmented layouts stall the DGE rings)
    fm_d = nc.dram_tensor("fm", [b_shard, P, CK, HW], F32R, kind="ExternalInput")
    lb_d = nc.dram_tensor("lb", [b_shard, P, LK, HW], F32R, kind="ExternalInput")
    wft_d = nc.dram_tensor("wft", [P, CK, D], F32R, kind="ExternalInput")
    wlt_d = nc.dram_tensor("wlt", [P, LK, D], F32R, kind="ExternalInput")
    wvt_d = nc.dram_tensor("wvt", [P, CK, C], BF16, kind="ExternalInput")
    bf_d = nc.dram_tensor("bfc", [1, D], F32R, kind="ExternalInput")
    bl_d = nc.dram_tensor("blc", [1, D], F32R, kind="ExternalInput")
    bv_d = nc.dram_tensor("bvc", [P, CM], F32, kind="ExternalInput")
    ones_d = nc.dram_tensor("ones", [1, P], F32R, kind="ExternalInput")
    out_d = nc.dram_tensor("out", [b_shard, P, CK, HW], F32, kind="ExternalOutput")

    with tile.TileContext(nc) as tc:
        with (
            tc.tile_pool(name="wpool", bufs=1) as wpool,
            tc.tile_pool(name="fmp", bufs=3) as fmp,
            tc.tile_pool(name="fmb", bufs=3) as fmb,
            tc.tile_pool(name="lbp", bufs=2) as lbp,
            tc.tile_pool(name="proj", bufs=2) as proj,
            tc.tile_pool(name="attp", bufs=2) as attp,
            tc.tile_pool(name="valp", bufs=1) as valp,
            tc.tile_pool(name="outp", bufs=4) as outp,
            tc.tile_pool(name="stat", bufs=2) as stat,
            tc.tile_pool(name="ps_small", bufs=3, space="PSUM") as ps_small,
            tc.tile_pool(name="ps_val", bufs=4, space="PSUM") as ps_val,
            tc.tile_pool(name="ps_att", bufs=1, space="PSUM") as ps_att,
        ):
            # ---- resident weights / constants ----
            wft = wpool.tile([P, CK, D], F32R)
            wlt = wpool.tile([P, LK, D], F32R)
            wvt = wpool.tile([P, CK, C], BF16)
            bfb = wpool.tile([1, D], F32R)
            blb = wpool.tile([1, D], F32R)
            bvb = wpool.tile([P, CM], F32)
            ones = wpool.tile([1, P], F32R)
            ident = wpool.tile([P, P], BF16)

            nc.sync.dma_start(ones[:], ones_d.ap())
            nc.sync.dma_start(bfb[:], bf_d.ap())
            nc.sync.dma_start(blb[:], bl_d.ap())
            nc.sync.dma_start(bvb[:], bv_d.ap())
            nc.sync.dma_start(wlt[:], wlt_d.ap())
            nc.sync.dma_start(wft[:], wft_d.ap())
            make_identity(nc, ident[:])

            fms = {}    # b -> (fm f32r tile, fm16 bf16 tile)
            atts = {}   # b -> attT tile

            def load(b):
                fm = fmp.tile([P, CK, HW], F32R)
                lbt = lbp.tile([P, LK, HW], F32R)
                fm16 = fmb.tile([P, CK, HW], BF16)
                nc.sync.dma_start(lbt[:], lb_d[b])
                # per-k-tile loads + casts so consumers track arrival
                for k in range(CK):
                    nc.sync.dma_start(fm[:, k, :], fm_d[b][:, k, :])
                    nc.vector.tensor_copy(fm16[:, k, :],
                                          fm[:, k, :].bitcast(F32))
                fms[b] = (fm, fm16)
                return lbt

            def att_path(b, lbt):
                """fkT/lqT/logits/softmax/transpose -> attT (PE: ~5us)."""
                fm = fms[b][0]
                lqt = proj.tile([P, NT, D], F32R, tag="lqt")
                for nt in range(NT):
                    ps = ps_small.tile([P, D], F32, tag="ps", name="ps")
                    for k in range(LK):
                        nc.tensor.matmul(
                            ps[:], lbt[:, k, nt * P:(nt + 1) * P], wlt[:, k, :],
                            start=(k == 0), stop=False)
                    nc.tensor.matmul(ps[:], ones[:], blb[:], start=False, stop=True)
                    nc.vector.tensor_copy(lqt[:, nt, :], ps[:])

                fkt = proj.tile([P, NT, D], F32R, tag="fkt")
                for nt in range(NT):
                    ps = ps_small.tile([P, D], F32, tag="ps", name="ps")
                    for k in range(CK):
                        nc.tensor.matmul(
                            ps[:], fm[:, k, nt * P:(nt + 1) * P], wft[:, k, :],
                            start=(k == 0), stop=False)
                    nc.tensor.matmul(ps[:], ones[:], bfb[:], start=False, stop=True)
                    nc.vector.tensor_copy(fkt[:, nt, :], ps[:])

                att = attp.tile([P, NT, D], BF16, tag="att")
                negmax = stat.tile([P, NT], F32, tag="negmax")
                sumexp = stat.tile([P, NT], F32, tag="sumexp")
                recip = stat.tile([P, NT], F32, tag="recip")
                for dm in range(NT):
                    ps = ps_small.tile([P, D], F32, tag="ps", name="ps")
                    for kn in range(NT):
                        nc.tensor.matmul(
                            ps[:], lqt[:, kn, dm * P:(dm + 1) * P], fkt[:, kn, :],
                            start=(kn == 0), stop=(kn == NT - 1))
                    nc.vector.tensor_reduce(
                        negmax[:, dm:dm + 1], ps[:], axis=mybir.AxisListType.X,
                        op=mybir.AluOpType.max, negate=True)
                    nc.scalar.activation(
                        att[:, dm, :], ps[:], mybir.ActivationFunctionType.Exp,
                        bias=negmax[:, dm:dm + 1], scale=1.0,
                        accum_out=sumexp[:, dm:dm + 1])
                    nc.vector.reciprocal(recip[:, dm:dm + 1], sumexp[:, dm:dm + 1])
                    nc.vector.tensor_scalar_mul(
                        att[:, dm, :], att[:, dm, :], recip[:, dm:dm + 1])

                attT = attp.tile([P, NT, D], BF16, tag="attT")
                for et in range(NT):
                    psT = ps_att.tile([P, D], BF16)
                    for dt_ in range(NT):
                        nc.tensor.transpose(
                            psT[:, dt_ * P:(dt_ + 1) * P],
                            att[:, dt_, et * P:(et + 1) * P], ident[:])
                    nc.scalar.copy(attT[:, et, :], psT[:])
                atts[b] = attT

            def final_pair(b, fm, vt, attT, cm, dma_eng=None):
                """out chunks cm, cm+1: matmuls + fused epilogue + one DMA."""
                dma_eng = dma_eng or nc.gpsimd
                ost = outp.tile([P, 2, D], F32, name="ost")
                for j in range(2):
                    ps = ps_small.tile([P, D], F32, tag="ps", name="fps")
                    for kn in range(NT):
                        nc.tensor.matmul(
                            ps[:], vt[:, kn, (cm + j) * P:(cm + j + 1) * P],
                            attT[:, kn, :],
                            start=(kn == 0), stop=(kn == NT - 1))
                    nc.vector.scalar_tensor_tensor(
                        ost[:, j, :], ps[:], bvb[:, cm + j:cm + j + 1],
                        fm[:, cm + j, :].bitcast(F32),
                        op0=mybir.AluOpType.add, op1=mybir.AluOpType.add)
                dma_eng.dma_start(out_d[b][:, cm:cm + 2, :], ost[:])

            def value_final(b):
                """vT (big GEMM) + out = vT.T @ attT + bv + residual."""
                fm, fm16 = fms.pop(b)
                attT = atts.pop(b)
                vt = valp.tile([P, NT, C], BF16, name="vt")
                for nt in range(NT):
                    # k-outer with 4 parallel PSUM banks: consumes wvt k-chunks
                    # in arrival order and shares each stationary fm16 tile
                    # across the 4 column-chunk matmuls.
                    pss = [ps_val.tile([P, VC], F32, tag="vps", name=f"vps{i}")
                           for i in range(C // VC)]
                    for k in range(CK):
                        for cc in range(C // VC):
                            nc.tensor.matmul(
                                pss[cc][:], fm16[:, k, nt * P:(nt + 1) * P],
                                wvt[:, k, cc * VC:(cc + 1) * VC],
                                start=(k == 0), stop=(k == CK - 1))
                    for cc in range(C // VC):
                        nc.scalar.copy(vt[:, nt, cc * VC:(cc + 1) * VC], pss[cc][:])

                for cm in range(0, CM, 2):
                    final_pair(b, fm, vt, attT, cm)

            def value_final_tail(b):
                """Last batch: cc-outer so each vT column chunk finishes early
                and its out chunks interleave with the remaining value GEMM,
                shrinking the serial tail."""
                fm, fm16 = fms.pop(b)
                attT = atts.pop(b)
                vt = valp.tile([P, NT, C], BF16, name="vt")
                for cc in range(C // VC):
                    pss = [ps_val.tile([P, VC], F32, tag="vps", name=f"vps{i}")
                           for i in range(NT)]
                    for k in range(CK):
                        for nt in range(NT):
                            nc.tensor.matmul(
                                pss[nt][:], fm16[:, k, nt * P:(nt + 1) * P],
                                wvt[:, k, cc * VC:(cc + 1) * VC],
                                start=(k == 0), stop=(k == CK - 1))
                    for nt in range(NT):
                        nc.scalar.copy(vt[:, nt, cc * VC:(cc + 1) * VC], pss[nt][:])
                    for cm in range(cc * (VC // P), (cc + 1) * (VC // P), 2):
                        final_pair(b, fm, vt, attT, cm, dma_eng=nc.sync)

            # software pipeline: attention path runs one batch ahead of the
            # big value GEMM so PE never waits on softmax, and the first
            # batches' attention work fills the initial weight-DMA window.
            lbt0 = load(0)
            lbt1 = load(1) if b_shard > 1 else None
            for k in range(CK):
                nc.gpsimd.dma_start(wvt[:, k, :], wvt_d.ap()[:, k, :])
            # warm the PE HAM clock gate during the initial weight-DMA window
            # (dummy rank-1 matmuls on the `ones` row; no real data needed)
            warm = ps_att.tile([P, P], F32, tag="psT", name="warm")
            for _ in range(80 if b_shard > 1 else 0):
                nc.tensor.matmul(warm[:], ones[:], ones[:], start=True, stop=True)
            att_path(0, lbt0)
            lbts = {1: lbt1}
            for b in range(b_shard):
                if b + 2 < b_shard:
                    lbts[b + 2] = load(b + 2)
                if b == b_shard - 1 and b_shard > 1:
                    value_final_tail(b)
                else:
                    value_final(b)
                if b + 1 < b_shard:
                    att_path(b + 1, lbts.pop(b + 1))

    nc.compile()
    return nc


_NC_CACHE = {}


def _get_nc(b_shard):
    if b_shard not in _NC_CACHE:
        _NC_CACHE[b_shard] = build_kernel(b_shard)
    return _NC_CACHE[b_shard]


def make_in_maps(feature_maps, labels, Wf, bf, Wl, bl, Wv, bv, b_shard=B_SHARD,
                 n_cores=N_CORES):
    def to_pkf(a, kt):
        # [rows=kt*P, free] -> [P, kt, free], partition-major for 1-line DMAs
        return np.ascontiguousarray(
            a.reshape(kt, P, a.shape[-1]).transpose(1, 0, 2))

    fm = np.asarray(feature_maps, dtype=np.float32).reshape(B, C, HW)
    fm = np.ascontiguousarray(
        fm.reshape(B, CK, P, HW).transpose(0, 2, 1, 3))
    lb = np.asarray(labels, dtype=np.float32).reshape(B, L, HW)
    lb = np.ascontiguousarray(
        lb.reshape(B, LK, P, HW).transpose(0, 2, 1, 3))
    wft = to_pkf(np.asarray(Wf, dtype=np.float32).T, CK)
    wlt = to_pkf(np.asarray(Wl, dtype=np.float32).T, LK)
    wvt = to_pkf(np.asarray(Wv, dtype=np.float32).T.astype(ml_dtypes.bfloat16),
                 CK)
    bfr = np.asarray(bf, dtype=np.float32).reshape(1, D)
    blr = np.asarray(bl, dtype=np.float32).reshape(1, D)
    bvr = np.ascontiguousarray(
        np.asarray(bv, dtype=np.float32).reshape(CM, P).T)
    in_maps = []
    for i in range(n_cores):
        s = slice(i * b_shard, (i + 1) * b_shard)
        in_maps.append({
            "fm": fm[s], "lb": lb[s], "wft": wft, "wlt": wlt, "wvt": wvt,
            "bfc": bfr, "blc": blr, "bvc": bvr,
            "ones": np.ones((1, P), dtype=np.float32),
        })
    return in_maps


def kernel(feature_maps, labels, Wf, bf, Wl, bl, Wv, bv, _trace=False,
           _tmpdir=None):
    nc = _get_nc(B_SHARD)
    in_maps = make_in_maps(feature_maps, labels, Wf, bf, Wl, bl, Wv, bv)
    res = run_bass_kernel_spmd(nc, in_maps, core_ids=list(range(N_CORES)),
                               trace=_trace, tmpdir=_tmpdir)
    out = np.concatenate([res.results[i]["out"] for i in range(N_CORES)], axis=0)
    kernel.last_exec_time_ns = res.exec_time_ns
    # [B, P, CK, HW] -> [B, C, H, W]
    out = out.transpose(0, 2, 1, 3).reshape(B, C, 16, 16)
    return np.ascontiguousarray(out).astype(np.float32)


# revision 16
# speedup vs baseline: 1.0109x; 1.0109x over previous
"""Cross-attention kernel for Trainium2, 8 NeuronCores, data-parallel over batch.

Reference computation (per batch b):
  lq = Wl @ lb + bl          [D, N]   (D == N == 256)
  fk = Wf @ fm + bf          [D, N]
  v  = Wv @ fm + bv          [C, N]
  att = softmax(lq @ fk.T)   [D, D]   (softmax over last dim)
  out = v @ att.T + fm       [C, N]

Strategy: batch 64 is split 8 ways (8 batches per core). All matmuls are
emitted with the activation tile as the stationary operand (lhsT) and the
weights streaming, computing transposed projections directly:
  fkT[n, d] = sum_c fm[c, n] WfT[c, d]     (fp32r / FP22)
  lqT[n, d] = sum_l lb[l, n] WlT[l, d]     (fp32r)
  logits[d, e] = sum_n lqT[n, d] fkT[n, e] (fp32r)
  vT[n, c] = sum_c' fm[c', n] WvT[c', c]   (bf16)
  out[c, d] = sum_n vT[n, c] attT[n, d]    (bf16)
Biases bl/bf enter via rank-1 augmentation matmuls (ones x bias row); bv is
added in the epilogue (exact: softmax rows sum to 1). Residual add is fused
into the PSUM->SBUF epilogue.
"""

import numpy as np
import ml_dtypes

import concourse.bass as bass
import concourse.mybir as mybir
import concourse.tile as tile
from concourse import bacc
from concourse.bass_utils import run_bass_kernel_spmd
from concourse.masks import make_identity

N_CORES = 8
B = 64
C = 2048
L = 512
HW = 256          # N = H*W, == D
D = HW
P = 128
B_SHARD = B // N_CORES

F32 = mybir.dt.float32
F32R = mybir.dt.float32r
BF16 = mybir.dt.bfloat16

CK = C // P       # 16 k-tiles over channel contraction
LK = L // P       # 4 k-tiles over label contraction
NT = HW // P      # 2 tiles over spatial/projection dim
CM = C // P       # 16 output-channel chunks
VC = 512          # value-matmul free-dim chunk (one PSUM bank of fp32)


def build_kernel(b_shard=B_SHARD):
    nc = bacc.Bacc("TRN2", target_bir_lowering=False, debug=False,
                   num_devices=N_CORES)

    # all tensors host-pre-transposed to [partition, ktile, free] so each DMA
    # is 128 large contiguous lines (descriptor-generation cost on the issuing
    # engine is ~2.4ns/line; fragmented layouts stall the DGE rings)
    fm_d = nc.dram_tensor("fm", [b_shard, P, CK, HW], F32R, kind="ExternalInput")
    lb_d = nc.dram_tensor("lb", [b_shard, P, LK, HW], F32R, kind="ExternalInput")
    wft_d = nc.dram_tensor("wft", [P, CK, D], F32R, kind="ExternalInput")
    wlt_d = nc.dram_tensor("wlt", [P, LK, D], F32R, kind="ExternalInput")
    wvt_d = nc.dram_tensor("wvt", [P, CK, C], BF16, kind="ExternalInput")
    bf_d = nc.dram_tensor("bfc", [1, D], F32R, kind="ExternalInput")
    bl_d = nc.dram_tensor("blc", [1, D], F32R, kind="ExternalInput")
    bv_d = nc.dram_tensor("bvc", [P, CM], F32, kind="ExternalInput")
    ones_d = nc.dram_tensor("ones", [1, P], F32R, kind="ExternalInput")
    out_d = nc.dram_tensor("out", [b_shard, P, CK, HW], F32, kind="ExternalOutput")

    with tile.TileContext(nc) as tc:
        with (
            tc.tile_pool(name="wpool", bufs=1) as wpool,
            tc.tile_pool(name="fmp", bufs=3) as fmp,
            tc.tile_pool(name="fmb", bufs=3) as fmb,
            tc.tile_pool(name="lbp", bufs=2) as lbp,
            tc.tile_pool(name="proj", bufs=2) as proj,
            tc.tile_pool(name="attp", bufs=2) as attp,
            tc.tile_pool(name="valp", bufs=1) as valp,
            tc.tile_pool(name="outp", bufs=4) as outp,
            tc.tile_pool(name="stat", bufs=2) as stat,
            tc.tile_pool(name="ps_small", bufs=3, space="PSUM") as ps_small,
            tc.tile_pool(name="ps_val", bufs=4, space="PSUM") as ps_val,
            tc.tile_pool(name="ps_att", bufs=1, space="PSUM") as ps_att,
        ):
            # ---- resident weights / constants ----
            wft = wpool.tile([P, CK, D], F32R)
            wlt = wpool.tile([P, LK, D], F32R)
            wvt = wpool.tile([P, CK, C], BF16)
            bfb = wpool.tile([1, D], F32R)
            blb = wpool.tile([1, D], F32R)
            bvb = wpool.tile([P, CM], F32)
            ones = wpool.tile([1, P], F32R)
            ident = wpool.tile([P, P], BF16)

            nc.sync.dma_start(ones[:], ones_d.ap())
            nc.sync.dma_start(bfb[:], bf_d.ap())
            nc.sync.dma_start(blb[:], bl_d.ap())
            nc.sync.dma_start(bvb[:], bv_d.ap())
            nc.sync.dma_start(wlt[:], wlt_d.ap())
            nc.sync.dma_start(wft[:], wft_d.ap())
            make_identity(nc, ident[:])

            fms = {}    # b -> (fm f32r tile, fm16 bf16 tile)
            atts = {}   # b -> attT tile

            def load(b):
                fm = fmp.tile([P, CK, HW], F32R)
                lbt = lbp.tile([P, LK, HW], F32R)
                fm16 = fmb.tile([P, CK, HW], BF16)
                nc.sync.dma_start(lbt[:], lb_d[b])
                nc.sync.dma_start(fm[:], fm_d[b])
                nc.vector.tensor_copy(fm16[:], fm[:].bitcast(F32))
                fms[b] = (fm, fm16)
                return lbt

            def att_path(b, lbt):
                """fkT/lqT/logits/softmax/transpose -> attT (PE: ~5us)."""
                fm = fms[b][0]
                lqt = proj.tile([P, NT, D], F32R, tag="lqt")
                for nt in range(NT):
                    ps = ps_small.tile([P, D], F32, tag="ps", name="ps")
                    for k in range(LK):
                        nc.tensor.matmul(
                            ps[:], lbt[:, k, nt * P:(nt + 1) * P], wlt[:, k, :],
                            start=(k == 0), stop=False)
                    nc.tensor.matmul(ps[:], ones[:], blb[:], start=False, stop=True)
                    nc.vector.tensor_copy(lqt[:, nt, :], ps[:])

                fkt = proj.tile([P, NT, D], F32R, tag="fkt")
                for nt in range(NT):
                    ps = ps_small.tile([P, D], F32, tag="ps", name="ps")
                    for k in range(CK):
                        nc.tensor.matmul(
                            ps[:], fm[:, k, nt * P:(nt + 1) * P], wft[:, k, :],
                            start=(k == 0), stop=False)
                    nc.tensor.matmul(ps[:], ones[:], bfb[:], start=False, stop=True)
                    nc.vector.tensor_copy(fkt[:, nt, :], ps[:])

                att = attp.tile([P, NT, D], BF16, tag="att")
                negmax = stat.tile([P, NT], F32, tag="negmax")
                sumexp = stat.tile([P, NT], F32, tag="sumexp")
                recip = stat.tile([P, NT], F32, tag="recip")
                for dm in range(NT):
                    ps = ps_small.tile([P, D], F32, tag="ps", name="ps")
                    for kn in range(NT):
                        nc.tensor.matmul(
                            ps[:], lqt[:, kn, dm * P:(dm + 1) * P], fkt[:, kn, :],
                            start=(kn == 0), stop=(kn == NT - 1))
                    nc.vector.tensor_reduce(
                        negmax[:, dm:dm + 1], ps[:], axis=mybir.AxisListType.X,
                        op=mybir.AluOpType.max, negate=True)
                    nc.scalar.activation(
                        att[:, dm, :], ps[:], mybir.ActivationFunctionType.Exp,
                        bias=negmax[:, dm:dm + 1], scale=1.0,
                        accum_out=sumexp[:, dm:dm + 1])
                    nc.vector.reciprocal(recip[:, dm:dm + 1], sumexp[:, dm:dm + 1])
                    nc.vector.tensor_scalar_mul(
                        att[:, dm, :], att[:, dm, :], recip[:, dm:dm + 1])

                attT = attp.tile([P, NT, D], BF16, tag="attT")
                for et in range(NT):
                    psT = ps_att.tile([P, D], BF16)
                    for dt_ in range(NT):
                        nc.tensor.transpose(
                            psT[:, dt_ * P:(dt_ + 1) * P],
                            att[:, dt_, et * P:(et + 1) * P], ident[:])
                    nc.scalar.copy(attT[:, et, :], psT[:])
                atts[b] = attT

            def final_pair(b, fm, vt, attT, cm, dma_eng=None):
                """out chunks cm, cm+1: matmuls + fused epilogue + one DMA."""
                dma_eng = dma_eng or nc.gpsimd
                ost = outp.tile([P, 2, D], F32, name="ost")
                for j in range(2):
                    ps = ps_small.tile([P, D], F32, tag="ps", name="fps")
                    for kn in range(NT):
                        nc.tensor.matmul(
                            ps[:], vt[:, kn, (cm + j) * P:(cm + j + 1) * P],
                            attT[:, kn, :],
                            start=(kn == 0), stop=(kn == NT - 1))
                    nc.vector.scalar_tensor_tensor(
                        ost[:, j, :], ps[:], bvb[:, cm + j:cm + j + 1],
                        fm[:, cm + j, :].bitcast(F32),
                        op0=mybir.AluOpType.add, op1=mybir.AluOpType.add)
                dma_eng.dma_start(out_d[b][:, cm:cm + 2, :], ost[:])

            def value_final(b):
                """vT (big GEMM) + out = vT.T @ attT + bv + residual."""
                fm, fm16 = fms.pop(b)
                attT = atts.pop(b)
                vt = valp.tile([P, NT, C], BF16, name="vt")
                for nt in range(NT):
                    # k-outer with 4 parallel PSUM banks: consumes wvt k-chunks
                    # in arrival order and shares each stationary fm16 tile
                    # across the 4 column-chunk matmuls.
                    pss = [ps_val.tile([P, VC], F32, tag="vps", name=f"vps{i}")
                           for i in range(C // VC)]
                    for k in range(CK):
                        for cc in range(C // VC):
                            nc.tensor.matmul(
                                pss[cc][:], fm16[:, k, nt * P:(nt + 1) * P],
                                wvt[:, k, cc * VC:(cc + 1) * VC],
                                start=(k == 0), stop=(k == CK - 1))
                    for cc in range(C // VC):
                        nc.scalar.copy(vt[:, nt, cc * VC:(cc + 1) * VC], pss[cc][:])

                for cm in range(0, CM, 2):
                    final_pair(b, fm, vt, attT, cm)

            def value_final_tail(b):
                """Last batch: cc-outer so each vT column chunk finishes early
                and its out chunks interleave with the remaining value GEMM,
                shrinking the serial tail."""
                fm, fm16 = fms.pop(b)
                attT = atts.pop(b)
                vt = valp.tile([P, NT, C], BF16, name="vt")
                for cc in range(C // VC):
                    pss = [ps_val.tile([P, VC], F32, tag="vps", name=f"vps{i}")
                           for i in range(NT)]
                    for k in range(CK):
                        for nt in range(NT):
                            nc.tensor.matmul(
                                pss[nt][:], fm16[:, k, nt * P:(nt + 1) * P],
                                wvt[:, k, cc * VC:(cc + 1) * VC],
                                start=(k == 0), stop=(k == CK - 1))
                    for nt in range(NT):
                        nc.scalar.copy(vt[:, nt, cc * VC:(cc + 1) * VC], pss[nt][:])
                    for cm in range(cc * (VC // P), (cc + 1) * (VC // P), 2):
                        final_pair(b, fm, vt, attT, cm, dma_eng=nc.sync)

            # software pipeline: attention path runs one batch ahead of the
            # big value GEMM so PE never waits on softmax, and the first
            # batches' attention work fills the initial weight-DMA window.
            lbt0 = load(0)
            lbt1 = load(1) if b_shard > 1 else None
            for k in range(CK):
                nc.gpsimd.dma_start(wvt[:, k, :], wvt_d.ap()[:, k, :])
            # warm the PE HAM clock gate during the initial weight-DMA window
            # (dummy rank-1 matmuls on the `ones` row; no real data needed)
            warm = ps_att.tile([P, P], F32, tag="psT", name="warm")
            for _ in range(120 if b_shard > 1 else 0):
                nc.tensor.matmul(warm[:], ones[:], ones[:], start=True, stop=True)
            att_path(0, lbt0)
            lbts = {1: lbt1}
            for b in range(b_shard):
                if b + 2 < b_shard:
                    lbts[b + 2] = load(b + 2)
                if b == b_shard - 1 and b_shard > 1:
                    value_final_tail(b)
                else:
                    value_final(b)
                if b + 1 < b_shard:
                    att_path(b + 1, lbts.pop(b + 1))

    nc.compile()
    return nc


_NC_CACHE = {}


def _get_nc(b_shard):
    if b_shard not in _NC_CACHE:
        _NC_CACHE[b_shard] = build_kernel(b_shard)
    return _NC_CACHE[b_shard]


def make_in_maps(feature_maps, labels, Wf, bf, Wl, bl, Wv, bv, b_shard=B_SHARD,
                 n_cores=N_CORES):
    def to_pkf(a, kt):
        # [rows=kt*P, free] -> [P, kt, free], partition-major for 1-line DMAs
        return np.ascontiguousarray(
            a.reshape(kt, P, a.shape[-1]).transpose(1, 0, 2))

    fm = np.asarray(feature_maps, dtype=np.float32).reshape(B, C, HW)
    fm = np.ascontiguousarray(
        fm.reshape(B, CK, P, HW).transpose(0, 2, 1, 3))
    lb = np.asarray(labels, dtype=np.float32).reshape(B, L, HW)
    lb = np.ascontiguousarray(
        lb.reshape(B, LK, P, HW).transpose(0, 2, 1, 3))
    wft = to_pkf(np.asarray(Wf, dtype=np.float32).T, CK)
    wlt = to_pkf(np.asarray(Wl, dtype=np.float32).T, LK)
    wvt = to_pkf(np.asarray(Wv, dtype=np.float32).T.astype(ml_dtypes.bfloat16),
                 CK)
    bfr = np.asarray(bf, dtype=np.float32).reshape(1, D)
    blr = np.asarray(bl, dtype=np.float32).reshape(1, D)
    bvr = np.ascontiguousarray(
        np.asarray(bv, dtype=np.float32).reshape(CM, P).T)
    in_maps = []
    for i in range(n_cores):
        s = slice(i * b_shard, (i + 1) * b_shard)
        in_maps.append({
            "fm": fm[s], "lb": lb[s], "wft": wft, "wlt": wlt, "wvt": wvt,
            "bfc": bfr, "blc": blr, "bvc": bvr,
            "ones": np.ones((1, P), dtype=np.float32),
        })
    return in_maps


def kernel(feature_maps, labels, Wf, bf, Wl, bl, Wv, bv, _trace=False,
           _tmpdir=None):
    nc = _get_nc(B_SHARD)
    in_maps = make_in_maps(feature_maps, labels, Wf, bf, Wl, bl, Wv, bv)
    res = run_bass_kernel_spmd(nc, in_maps, core_ids=list(range(N_CORES)),
                               trace=_trace, tmpdir=_tmpdir)
    out = np.concatenate([res.results[i]["out"] for i in range(N_CORES)], axis=0)
    kernel.last_exec_time_ns = res.exec_time_ns
    # [B, P, CK, HW] -> [B, C, H, W]
    out = out.transpose(0, 2, 1, 3).reshape(B, C, 16, 16)
    return np.ascontiguousarray(out).astype(np.float32)


# revision 17
# speedup vs baseline: 1.0307x; 1.0196x over previous
"""Cross-attention kernel for Trainium2, 8 NeuronCores, data-parallel over batch.

Reference computation (per batch b):
  lq = Wl @ lb + bl          [D, N]   (D == N == 256)
  fk = Wf @ fm + bf          [D, N]
  v  = Wv @ fm + bv          [C, N]
  att = softmax(lq @ fk.T)   [D, D]   (softmax over last dim)
  out = v @ att.T + fm       [C, N]

Strategy: batch 64 is split 8 ways (8 batches per core). All matmuls are
emitted with the activation tile as the stationary operand (lhsT) and the
weights streaming, computing transposed projections directly:
  fkT[n, d] = sum_c fm[c, n] WfT[c, d]     (fp32r / FP22)
  lqT[n, d] = sum_l lb[l, n] WlT[l, d]     (fp32r)
  logits[d, e] = sum_n lqT[n, d] fkT[n, e] (fp32r)
  vT[n, c] = sum_c' fm[c', n] WvT[c', c]   (bf16)
  out[c, d] = sum_n vT[n, c] attT[n, d]    (bf16)
Biases bl/bf enter via rank-1 augmentation matmuls (ones x bias row); bv is
added in the epilogue (exact: softmax rows sum to 1). Residual add is fused
into the PSUM->SBUF epilogue.
"""

import numpy as np
import ml_dtypes

import concourse.bass as bass
import concourse.mybir as mybir
import concourse.tile as tile
from concourse import bacc
from concourse.bass_utils import run_bass_kernel_spmd
from concourse.masks import make_identity

N_CORES = 8
B = 64
C = 2048
L = 512
HW = 256          # N = H*W, == D
D = HW
P = 128
B_SHARD = B // N_CORES

F32 = mybir.dt.float32
F32R = mybir.dt.float32r
BF16 = mybir.dt.bfloat16

CK = C // P       # 16 k-tiles over channel contraction
LK = L // P       # 4 k-tiles over label contraction
NT = HW // P      # 2 tiles over spatial/projection dim
CM = C // P       # 16 output-channel chunks
VC = 512          # value-matmul free-dim chunk (one PSUM bank of fp32)


def build_kernel(b_shard=B_SHARD):
    nc = bacc.Bacc("TRN2", target_bir_lowering=False, debug=False,
                   num_devices=N_CORES)

    # all tensors host-pre-transposed to [partition, ktile, free] so each DMA
    # is 128 large contiguous lines (descriptor-generation cost on the issuing
    # engine is ~2.4ns/line; fragmented layouts stall the DGE rings)
    fm_d = nc.dram_tensor("fm", [b_shard, P, CK, HW], F32R, kind="ExternalInput")
    lb_d = nc.dram_tensor("lb", [b_shard, P, LK, HW], F32R, kind="ExternalInput")
    wft_d = nc.dram_tensor("wft", [P, CK, D], F32R, kind="ExternalInput")
    wlt_d = nc.dram_tensor("wlt", [P, LK, D], F32R, kind="ExternalInput")
    wvt_d = nc.dram_tensor("wvt", [P, CK, C], BF16, kind="ExternalInput")
    bf_d = nc.dram_tensor("bfc", [1, D], F32R, kind="ExternalInput")
    bl_d = nc.dram_tensor("blc", [1, D], F32R, kind="ExternalInput")
    bv_d = nc.dram_tensor("bvc", [P, CM], F32, kind="ExternalInput")
    ones_d = nc.dram_tensor("ones", [1, P], F32R, kind="ExternalInput")
    out_d = nc.dram_tensor("out", [b_shard, P, CK, HW], F32, kind="ExternalOutput")

    with tile.TileContext(nc) as tc:
        with (
            tc.tile_pool(name="wpool", bufs=1) as wpool,
            tc.tile_pool(name="fmp", bufs=2) as fmp,
            tc.tile_pool(name="fmb", bufs=2) as fmb,
            tc.tile_pool(name="lbp", bufs=2) as lbp,
            tc.tile_pool(name="proj", bufs=2) as proj,
            tc.tile_pool(name="attp", bufs=2) as attp,
            tc.tile_pool(name="valp", bufs=1) as valp,
            tc.tile_pool(name="outp", bufs=4) as outp,
            tc.tile_pool(name="stat", bufs=2) as stat,
            tc.tile_pool(name="ps_small", bufs=3, space="PSUM") as ps_small,
            tc.tile_pool(name="ps_val", bufs=4, space="PSUM") as ps_val,
            tc.tile_pool(name="ps_att", bufs=1, space="PSUM") as ps_att,
        ):
            # ---- resident weights / constants ----
            wft = wpool.tile([P, CK, D], F32R)
            wlt = wpool.tile([P, LK, D], F32R)
            wvt = wpool.tile([P, CK, C], BF16)
            bfb = wpool.tile([1, D], F32R)
            blb = wpool.tile([1, D], F32R)
            bvb = wpool.tile([P, CM], F32)
            ones = wpool.tile([1, P], F32R)
            ident = wpool.tile([P, P], BF16)

            nc.sync.dma_start(ones[:], ones_d.ap())
            nc.sync.dma_start(bfb[:], bf_d.ap())
            nc.sync.dma_start(blb[:], bl_d.ap())
            nc.sync.dma_start(bvb[:], bv_d.ap())
            nc.sync.dma_start(wlt[:], wlt_d.ap())
            nc.sync.dma_start(wft[:], wft_d.ap())
            make_identity(nc, ident[:])

            fms = {}    # b -> (fm f32r tile, fm16 bf16 tile)
            atts = {}   # b -> attT tile

            def load(b):
                fm = fmp.tile([P, CK, HW], F32R)
                lbt = lbp.tile([P, LK, HW], F32R)
                fm16 = fmb.tile([P, CK, HW], BF16)
                nc.sync.dma_start(lbt[:], lb_d[b])
                nc.sync.dma_start(fm[:], fm_d[b])
                nc.vector.tensor_copy(fm16[:], fm[:].bitcast(F32))
                fms[b] = (fm, fm16)
                return lbt

            def att_path(b, lbt):
                """fkT/lqT/logits/softmax/transpose -> attT (PE: ~5us)."""
                fm = fms[b][0]
                lqt = proj.tile([P, NT, D], F32R, tag="lqt")
                for nt in range(NT):
                    ps = ps_small.tile([P, D], F32, tag="ps", name="ps")
                    for k in range(LK):
                        nc.tensor.matmul(
                            ps[:], lbt[:, k, nt * P:(nt + 1) * P], wlt[:, k, :],
                            start=(k == 0), stop=False)
                    nc.tensor.matmul(ps[:], ones[:], blb[:], start=False, stop=True)
                    nc.vector.tensor_copy(lqt[:, nt, :], ps[:])

                fkt = proj.tile([P, NT, D], F32R, tag="fkt")
                for nt in range(NT):
                    ps = ps_small.tile([P, D], F32, tag="ps", name="ps")
                    for k in range(CK):
                        nc.tensor.matmul(
                            ps[:], fm[:, k, nt * P:(nt + 1) * P], wft[:, k, :],
                            start=(k == 0), stop=False)
                    nc.tensor.matmul(ps[:], ones[:], bfb[:], start=False, stop=True)
                    nc.vector.tensor_copy(fkt[:, nt, :], ps[:])

                att = attp.tile([P, NT, D], BF16, tag="att")
                negmax = stat.tile([P, NT], F32, tag="negmax")
                sumexp = stat.tile([P, NT], F32, tag="sumexp")
                recip = stat.tile([P, NT], F32, tag="recip")
                for dm in range(NT):
                    ps = ps_small.tile([P, D], F32, tag="ps", name="ps")
                    for kn in range(NT):
                        nc.tensor.matmul(
                            ps[:], lqt[:, kn, dm * P:(dm + 1) * P], fkt[:, kn, :],
                            start=(kn == 0), stop=(kn == NT - 1))
                    nc.vector.tensor_reduce(
                        negmax[:, dm:dm + 1], ps[:], axis=mybir.AxisListType.X,
                        op=mybir.AluOpType.max, negate=True)
                    nc.scalar.activation(
                        att[:, dm, :], ps[:], mybir.ActivationFunctionType.Exp,
                        bias=negmax[:, dm:dm + 1], scale=1.0,
                        accum_out=sumexp[:, dm:dm + 1])
                    nc.vector.reciprocal(recip[:, dm:dm + 1], sumexp[:, dm:dm + 1])
                    nc.vector.tensor_scalar_mul(
                        att[:, dm, :], att[:, dm, :], recip[:, dm:dm + 1])

                attT = attp.tile([P, NT, D], BF16, tag="attT")
                for et in range(NT):
                    psT = ps_att.tile([P, D], BF16)
                    for dt_ in range(NT):
                        nc.tensor.transpose(
                            psT[:, dt_ * P:(dt_ + 1) * P],
                            att[:, dt_, et * P:(et + 1) * P], ident[:])
                    nc.scalar.copy(attT[:, et, :], psT[:])
                atts[b] = attT

            def final_pair(b, fm, vt, attT, cm, dma_eng=None):
                """out chunks cm, cm+1: matmuls + fused epilogue + one DMA."""
                dma_eng = dma_eng or nc.gpsimd
                ost = outp.tile([P, 2, D], F32, name="ost")
                for j in range(2):
                    ps = ps_small.tile([P, D], F32, tag="ps", name="fps")
                    for kn in range(NT):
                        nc.tensor.matmul(
                            ps[:], vt[:, kn, (cm + j) * P:(cm + j + 1) * P],
                            attT[:, kn, :],
                            start=(kn == 0), stop=(kn == NT - 1))
                    nc.vector.scalar_tensor_tensor(
                        ost[:, j, :], ps[:], bvb[:, cm + j:cm + j + 1],
                        fm[:, cm + j, :].bitcast(F32),
                        op0=mybir.AluOpType.add, op1=mybir.AluOpType.add)
                dma_eng.dma_start(out_d[b][:, cm:cm + 2, :], ost[:])

            def value_final(b):
                """vT (big GEMM) + out = vT.T @ attT + bv + residual."""
                fm, fm16 = fms.pop(b)
                attT = atts.pop(b)
                vt = valp.tile([P, NT, C], BF16, name="vt")
                for nt in range(NT):
                    # k-outer with 4 parallel PSUM banks: consumes wvt k-chunks
                    # in arrival order and shares each stationary fm16 tile
                    # across the 4 column-chunk matmuls.
                    pss = [ps_val.tile([P, VC], F32, tag="vps", name=f"vps{i}")
                           for i in range(C // VC)]
                    for k in range(CK):
                        for cc in range(C // VC):
                            nc.tensor.matmul(
                                pss[cc][:], fm16[:, k, nt * P:(nt + 1) * P],
                                wvt[:, k, cc * VC:(cc + 1) * VC],
                                start=(k == 0), stop=(k == CK - 1))
                    for cc in range(C // VC):
                        nc.scalar.copy(vt[:, nt, cc * VC:(cc + 1) * VC], pss[cc][:])

                for cm in range(0, CM, 2):
                    final_pair(b, fm, vt, attT, cm)

            def value_final_tail(b):
                """Last batch: cc-outer so each vT column chunk finishes early
                and its out chunks interleave with the remaining value GEMM,
                shrinking the serial tail."""
                fm, fm16 = fms.pop(b)
                attT = atts.pop(b)
                vt = valp.tile([P, NT, C], BF16, name="vt")
                for cc in range(C // VC):
                    pss = [ps_val.tile([P, VC], F32, tag="vps", name=f"vps{i}")
                           for i in range(NT)]
                    for k in range(CK):
                        for nt in range(NT):
                            nc.tensor.matmul(
                                pss[nt][:], fm16[:, k, nt * P:(nt + 1) * P],
                                wvt[:, k, cc * VC:(cc + 1) * VC],
                                start=(k == 0), stop=(k == CK - 1))
                    for nt in range(NT):
                        nc.scalar.copy(vt[:, nt, cc * VC:(cc + 1) * VC], pss[nt][:])
                    for cm in range(cc * (VC // P), (cc + 1) * (VC // P), 2):
                        final_pair(b, fm, vt, attT, cm, dma_eng=nc.sync)

            # software pipeline: attention path runs one batch ahead of the
            # big value GEMM so PE never waits on softmax, and the first
            # batches' attention work fills the initial weight-DMA window.
            lbt0 = load(0)
            for k in range(CK):
                nc.gpsimd.dma_start(wvt[:, k, :], wvt_d.ap()[:, k, :])
            # warm the PE HAM clock gate during the initial weight-DMA window.
            # K=128 matmuls on the identity tile keep the full array active
            # (K=1 dummies do not register as PE-busy for the HAM).
            warm = ps_att.tile([P, P], F32, tag="psT", name="warm")
            for _ in range(230 if b_shard > 1 else 0):
                nc.tensor.matmul(warm[:], ident[:], ident[:, :P], start=True,
                                 stop=True)
            att_path(0, lbt0)
            for b in range(b_shard):
                if b + 1 < b_shard:
                    lbt = load(b + 1)
                if b == b_shard - 1 and b_shard > 1:
                    value_final_tail(b)
                else:
                    value_final(b)
                if b + 1 < b_shard:
                    att_path(b + 1, lbt)

    nc.compile()
    return nc


_NC_CACHE = {}


def _get_nc(b_shard):
    if b_shard not in _NC_CACHE:
        _NC_CACHE[b_shard] = build_kernel(b_shard)
    return _NC_CACHE[b_shard]


def make_in_maps(feature_maps, labels, Wf, bf, Wl, bl, Wv, bv, b_shard=B_SHARD,
                 n_cores=N_CORES):
    def to_pkf(a, kt):
        # [rows=kt*P, free] -> [P, kt, free], partition-major for 1-line DMAs
        return np.ascontiguousarray(
            a.reshape(kt, P, a.shape[-1]).transpose(1, 0, 2))

    fm = np.asarray(feature_maps, dtype=np.float32).reshape(B, C, HW)
    fm = np.ascontiguousarray(
        fm.reshape(B, CK, P, HW).transpose(0, 2, 1, 3))
    lb = np.asarray(labels, dtype=np.float32).reshape(B, L, HW)
    lb = np.ascontiguousarray(
        lb.reshape(B, LK, P, HW).transpose(0, 2, 1, 3))
    wft = to_pkf(np.asarray(Wf, dtype=np.float32).T, CK)
    wlt = to_pkf(np.asarray(Wl, dtype=np.float32).T, LK)
    wvt = to_pkf(np.asarray(Wv, dtype=np.float32).T.astype(ml_dtypes.bfloat16),
                 CK)
    bfr = np.asarray(bf, dtype=np.float32).reshape(1, D)
    blr = np.asarray(bl, dtype=np.float32).reshape(1, D)
    bvr = np.ascontiguousarray(
        np.asarray(bv, dtype=np.float32).reshape(CM, P).T)
    in_maps = []
    for i in range(n_cores):
        s = slice(i * b_shard, (i + 1) * b_shard)
        in_maps.append({
            "fm": fm[s], "lb": lb[s], "wft": wft, "wlt": wlt, "wvt": wvt,
            "bfc": bfr, "blc": blr, "bvc": bvr,
            "ones": np.ones((1, P), dtype=np.float32),
        })
    return in_maps


def kernel(feature_maps, labels, Wf, bf, Wl, bl, Wv, bv, _trace=False,
           _tmpdir=None):
    nc = _get_nc(B_SHARD)
    in_maps = make_in_maps(feature_maps, labels, Wf, bf, Wl, bl, Wv, bv)
    res = run_bass_kernel_spmd(nc, in_maps, core_ids=list(range(N_CORES)),
                               trace=_trace, tmpdir=_tmpdir)
    out = np.concatenate([res.results[i]["out"] for i in range(N_CORES)], axis=0)
    kernel.last_exec_time_ns = res.exec_time_ns
    # [B, P, CK, HW] -> [B, C, H, W]
    out = out.transpose(0, 2, 1, 3).reshape(B, C, 16, 16)
    return np.ascontiguousarray(out).astype(np.float32)
